# revision 27
# baseline (speedup 1.0000x reference)
"""Batched structure decoder: out[g] = sigmoid(z_g @ z_g^T), masked to valid nodes.

Full inputs in, full output out. Shards the 128 graphs across 8 NeuronCores
(16 graphs each); each core computes its own [16, 512, 512] block with no
cross-device communication.

Per-core device kernel (Bass/Tile), HBM-bandwidth-shaped:
  - All 16 input reads (z fp32) are hoisted to the front of the sync HWDGE
    ring so the read phase physically completes before the write phase
    starts (mixed read+write HBM traffic measured ~25% slower than
    phase-separated).
  - fp32 -> fp16 casts and PSUM->SBUF transpose copies run on DVE; the
    z32 staging pool holds all 16 graphs so reads never wait on compute.
  - Per graph: 8 fp16 PE transposes (1 cycle/row) build zT, 8 fp16 matmuls
    (1 cycle/row) accumulate into fp32 PSUM, ScalarE applies sigmoid.
  - Output is written as fp16 (sigmoid is in [0,1]; abs error <= 2.4e-4)
    which halves the write traffic; the host casts back to fp32.
"""

import numpy as np

import concourse.bass as bass
import concourse.tile as tile
from concourse import bacc, mybir
from concourse.bass_utils import run_bass_kernel_spmd
from concourse.masks import make_identity

NUM_GRAPHS = 128
MAX_NODES = 512
LATENT_DIM = 256
N_CORES = 8
G_PER_CORE = NUM_GRAPHS // N_CORES  # 16
P = 128
N_TILES = MAX_NODES // P  # 4 node tiles per graph
K_TILES = LATENT_DIM // P  # 2 contraction subtiles

_NC = None  # cached Bass program
_last_results = None  # BassKernelResults of the most recent run (for profiling)


def _build_bass():
    nc = bacc.Bacc("TRN2", target_bir_lowering=False)
    z = nc.dram_tensor(
        "z", (G_PER_CORE * MAX_NODES, LATENT_DIM), mybir.dt.float32,
        kind="ExternalInput",
    )
    out = nc.dram_tensor(
        "out", (G_PER_CORE, MAX_NODES, MAX_NODES), mybir.dt.float16,
        kind="ExternalOutput",
    )
    # z[g*512 + t*128 + p, d] -> [g, p, t, d]
    z_r = z[:].rearrange("(g t p) d -> g p t d", t=N_TILES, p=P)
    # Output rows are computed even/odd interleaved: within each 256-row
    # block b, PSUM partition p holds rows 256b + 2p (even matmul) and
    # 256b + 2p + 1 (odd matmul), so each partition's slice of the output
    # DMA is 2 KB contiguous (vs 1 KB row-sized descriptors otherwise --
    # measured ~15% better HBM write throughput).
    # out[g, 256b + 2p + e, n] -> [g, p, b, (e n)]
    out_r = out[:].rearrange("g (b p e) n -> g p b (e n)", b=2, e=2)

    with tile.TileContext(nc) as tc:
        with (
            tc.tile_pool(name="singles", bufs=1) as singles,
            tc.tile_pool(name="zin", bufs=G_PER_CORE) as zin_pool,
            tc.tile_pool(name="z32", bufs=G_PER_CORE) as z32_pool,
            tc.tile_pool(name="zt", bufs=6) as zt_pool,
            tc.tile_pool(name="osb", bufs=14) as out_pool,
            tc.tile_pool(name="pst", bufs=2, space="PSUM") as psum_t_pool,
            tc.tile_pool(name="psmm", bufs=3, space="PSUM") as psum_mm_pool,
        ):
            identity = singles.tile([P, P], mybir.dt.float16)
            make_identity(nc, identity)

            # Prewarm the ACT sigmoid table (ACT_TABLE_LOAD + DRAIN ~2.7us)
            # during the read phase so the first real sigmoid isn't blocked.
            warm = singles.tile([P, 1], mybir.dt.float32)
            nc.vector.memset(warm, 0.0)
            nc.scalar.activation(
                out=warm, in_=warm, func=mybir.ActivationFunctionType.Sigmoid
            )

            # Prewarm the PE HAM clock gate: ~3.5us of dummy transposes during
            # the read phase flips the PE clock from 1.2 to 2.4 GHz before the
            # first real matmuls arrive (otherwise the pipeline fill runs at
            # half speed). Shares the ps_t tag so no extra PSUM banks.
            warm_ps = psum_t_pool.tile(
                [P, K_TILES, MAX_NODES], mybir.dt.float16, tag="ps_t"
            )
            for _ in range(32):
                nc.tensor.transpose(warm_ps[:, 0, 0:P], identity, identity)

            # Read phase: all input DMAs first on the sync ring (per-engine
            # FIFO => reads complete before the first output write starts).
            # z32 staging holds every graph, so no read ever waits on a
            # slot-release from compute.
            z32_all = []
            for g in range(G_PER_CORE):
                z32 = z32_pool.tile([P, N_TILES, LATENT_DIM], mybir.dt.float32)
                nc.sync.dma_start(out=z32, in_=z_r[g])
                z32_all.append(z32)

            for g in range(G_PER_CORE):
                # fp32 -> fp16 cast on DVE, in-loop so it interleaves with the
                # zT copies below on the same engine queue.
                z16 = zin_pool.tile([P, N_TILES, LATENT_DIM], mybir.dt.float16)
                nc.vector.tensor_copy(out=z16, in_=z32_all[g])

                # Transpose to zT[p=d % 128, kt, n] (fp16, 1 cycle/row on PE).
                # All 8 transposes of one graph land in ONE psum bank (fp16
                # [128, 2*512] = 2KB/partition); one DVE copy moves them out.
                zT = zt_pool.tile([P, K_TILES, MAX_NODES], mybir.dt.float16)
                ps_t = psum_t_pool.tile([P, K_TILES, MAX_NODES], mybir.dt.float16)
                for kt in range(K_TILES):
                    for t in range(N_TILES):
                        nc.tensor.transpose(
                            ps_t[:, kt, t * P:(t + 1) * P],
                            z16[:, t, kt * P:(kt + 1) * P],
                            identity,
                        )
                nc.vector.tensor_copy(
                    out=zT.rearrange("p k n -> p (k n)"),
                    in_=ps_t.rearrange("p k n -> p (k n)"),
                )

                # Two 256-row blocks b, each computed as an (even, odd) pair
                # of matmuls whose lhsT picks alternating zT columns =>
                # [128, 1024] PSUM tiles; psum partition p covers output rows
                # 256b + 2p and 256b + 2p + 1.
                for b in range(2):
                    mm_ps = psum_mm_pool.tile([P, 2 * MAX_NODES], mybir.dt.float32)
                    for eo in range(2):
                        lhsT_cols = zT[:, :, 2 * b * P + eo:2 * (b + 1) * P:2]
                        for kt in range(K_TILES):
                            nc.tensor.matmul(
                                mm_ps[:, eo * MAX_NODES:(eo + 1) * MAX_NODES],
                                lhsT=lhsT_cols[:, kt, :],
                                rhs=zT[:, kt, :],
                                start=(kt == 0),
                                stop=(kt == K_TILES - 1),
                            )
                    o_t = out_pool.tile([P, 2 * MAX_NODES], mybir.dt.float16)
                    nc.scalar.activation(
                        out=o_t,
                        in_=mm_ps,
                        func=mybir.ActivationFunctionType.Sigmoid,
                    )
                    nc.sync.dma_start(out=out_r[g, :, b], in_=o_t)

    nc.compile()
    return nc


def _get_nc():
    global _NC
    if _NC is None:
        _NC = _build_bass()
    return _NC


def kernel(z, batch, num_graphs, max_nodes):
    global _last_results
    z = np.ascontiguousarray(np.asarray(z), dtype=np.float32)
    batch = np.asarray(batch)
    G = int(num_graphs)
    N = int(max_nodes)
    n_total, d = z.shape
    assert (G, N, d, n_total) == (NUM_GRAPHS, MAX_NODES, LATENT_DIM,
                                  NUM_GRAPHS * MAX_NODES), "hardcoded shapes"

    # Fast path: every graph has exactly max_nodes contiguous nodes.
    expected_batch = (np.arange(n_total) // N).astype(batch.dtype)
    dense = np.array_equal(batch, expected_batch)
    if dense:
        z_full = z
        mask2d = None
    else:
        # General ragged path: scatter into zero-padded [G, N, d] on host,
        # run the same device kernel, then zero out masked positions.
        counts = np.bincount(batch, minlength=G)
        starts = np.concatenate([[0], np.cumsum(counts)[:-1]])
        pos = np.arange(n_total) - starts[batch]
        z_pad = np.zeros((G, N, d), np.float32)
        valid = np.zeros((G, N), bool)
        z_pad[batch, pos] = z
        valid[batch, pos] = True
        z_full = z_pad.reshape(G * N, d)
        mask2d = valid[:, :, None] & valid[:, None, :]

    nc = _get_nc()
    rows = G_PER_CORE * MAX_NODES
    in_maps = [
        {"z": z_full[c * rows:(c + 1) * rows]} for c in range(N_CORES)
    ]
    _last_results = run_bass_kernel_spmd(
        nc, in_maps, core_ids=list(range(N_CORES))
    )
    out = np.concatenate(
        [r["out"] for r in _last_results.results], axis=0
    ).astype(np.float32)

    if mask2d is not None:
        out = np.where(mask2d, out, np.float32(0.0))
    return out


# revision 28
# speedup vs baseline: 1.0277x; 1.0277x over previous
"""Batched structure decoder: out[g] = sigmoid(z_g @ z_g^T), masked to valid nodes.

Full inputs in, full output out. Shards the 128 graphs across 8 NeuronCores
(16 graphs each); each core computes its own [16, 512, 512] block with no
cross-device communication.

Per-core device kernel (Bass/Tile), HBM-bandwidth-shaped:
  - All 16 input reads (z fp32) are hoisted to the front of the sync HWDGE
    ring so the read phase physically completes before the write phase
    starts (mixed read+write HBM traffic measured ~25% slower than
    phase-separated).
  - fp32 -> fp16 casts and PSUM->SBUF transpose copies run on DVE; the
    z32 staging pool holds all 16 graphs so reads never wait on compute.
  - Per graph: 8 fp16 PE transposes (1 cycle/row) build zT, 8 fp16 matmuls
    (1 cycle/row) accumulate into fp32 PSUM, ScalarE applies sigmoid.
  - Output is written as fp16 (sigmoid is in [0,1]; abs error <= 2.4e-4)
    which halves the write traffic; the host casts back to fp32.
"""

import numpy as np

import concourse.bass as bass
import concourse.tile as tile
from concourse import bacc, mybir
from concourse.bass_utils import run_bass_kernel_spmd
from concourse.masks import make_identity

NUM_GRAPHS = 128
MAX_NODES = 512
LATENT_DIM = 256
N_CORES = 8
G_PER_CORE = NUM_GRAPHS // N_CORES  # 16
P = 128
N_TILES = MAX_NODES // P  # 4 node tiles per graph
K_TILES = LATENT_DIM // P  # 2 contraction subtiles

_NC = None  # cached Bass program
_last_results = None  # BassKernelResults of the most recent run (for profiling)


def _build_bass():
    nc = bacc.Bacc("TRN2", target_bir_lowering=False)
    z = nc.dram_tensor(
        "z", (G_PER_CORE * MAX_NODES, LATENT_DIM), mybir.dt.float32,
        kind="ExternalInput",
    )
    out = nc.dram_tensor(
        "out", (G_PER_CORE, MAX_NODES, MAX_NODES), mybir.dt.float16,
        kind="ExternalOutput",
    )
    # z[g*512 + t*128 + p, d] -> [g, p, t, d]
    z_r = z[:].rearrange("(g t p) d -> g p t d", t=N_TILES, p=P)
    # Output rows are computed even/odd interleaved: within each 256-row
    # block b, PSUM partition p holds rows 256b + 2p (even matmul) and
    # 256b + 2p + 1 (odd matmul), so each partition's slice of the output
    # DMA is 2 KB contiguous (vs 1 KB row-sized descriptors otherwise --
    # measured ~15% better HBM write throughput).
    # out[g, 256b + 2p + e, n] -> [g, p, b, (e n)]
    out_r = out[:].rearrange("g (b p e) n -> g p b (e n)", b=2, e=2)

    with tile.TileContext(nc) as tc:
        with (
            tc.tile_pool(name="singles", bufs=1) as singles,
            tc.tile_pool(name="zin", bufs=G_PER_CORE) as zin_pool,
            tc.tile_pool(name="z32", bufs=G_PER_CORE) as z32_pool,
            tc.tile_pool(name="zt", bufs=6) as zt_pool,
            tc.tile_pool(name="osb", bufs=14) as out_pool,
            tc.tile_pool(name="pst", bufs=2, space="PSUM") as psum_t_pool,
            tc.tile_pool(name="psmm", bufs=3, space="PSUM") as psum_mm_pool,
        ):
            identity = singles.tile([P, P], mybir.dt.float16)
            make_identity(nc, identity)

            # Prewarm the ACT sigmoid table (ACT_TABLE_LOAD + DRAIN ~2.7us)
            # during the read phase so the first real sigmoid isn't blocked.
            warm = singles.tile([P, 1], mybir.dt.float32)
            nc.vector.memset(warm, 0.0)
            nc.scalar.activation(
                out=warm, in_=warm, func=mybir.ActivationFunctionType.Sigmoid
            )

            # Prewarm the PE HAM clock gate: ~3.5us of dummy transposes during
            # the read phase flips the PE clock from 1.2 to 2.4 GHz before the
            # first real matmuls arrive (otherwise the pipeline fill runs at
            # half speed). Shares the ps_t tag so no extra PSUM banks.
            warm_ps = psum_t_pool.tile(
                [P, K_TILES, MAX_NODES], mybir.dt.float16, tag="ps_t"
            )
            for _ in range(32):
                nc.tensor.transpose(warm_ps[:, 0, 0:P], identity, identity)

            # Read phase: all input DMAs first on the sync ring (per-engine
            # FIFO => reads complete before the first output write starts).
            # z32 staging holds every graph, so no read ever waits on a
            # slot-release from compute.
            z32_all = []
            for g in range(G_PER_CORE):
                z32 = z32_pool.tile([P, N_TILES, LATENT_DIM], mybir.dt.float32)
                nc.sync.dma_start(out=z32, in_=z_r[g])
                z32_all.append(z32)

            for g in range(G_PER_CORE):
                # fp32 -> fp16 cast on DVE, in-loop so it interleaves with the
                # zT copies below on the same engine queue.
                z16 = zin_pool.tile([P, N_TILES, LATENT_DIM], mybir.dt.float16)
                nc.vector.tensor_copy(out=z16, in_=z32_all[g])

                # Transpose to zT[p=d % 128, kt, n] (fp16, 1 cycle/row on PE).
                # All 8 transposes of one graph land in ONE psum bank (fp16
                # [128, 2*512] = 2KB/partition); one DVE copy moves them out.
                zT = zt_pool.tile([P, K_TILES, MAX_NODES], mybir.dt.float16)
                ps_t = psum_t_pool.tile([P, K_TILES, MAX_NODES], mybir.dt.float16)
                for kt in range(K_TILES):
                    for t in range(N_TILES):
                        nc.tensor.transpose(
                            ps_t[:, kt, t * P:(t + 1) * P],
                            z16[:, t, kt * P:(kt + 1) * P],
                            identity,
                        )
                nc.vector.tensor_copy(
                    out=zT.rearrange("p k n -> p (k n)"),
                    in_=ps_t.rearrange("p k n -> p (k n)"),
                )

                # Two 256-row blocks b, each computed as an (even, odd) pair
                # of matmuls whose lhsT picks alternating zT columns =>
                # [128, 1024] PSUM tiles; psum partition p covers output rows
                # 256b + 2p and 256b + 2p + 1.
                o_t = out_pool.tile([P, 2, 2 * MAX_NODES], mybir.dt.float16)
                for b in range(2):
                    mm_ps = psum_mm_pool.tile([P, 2 * MAX_NODES], mybir.dt.float32)
                    for eo in range(2):
                        lhsT_cols = zT[:, :, 2 * b * P + eo:2 * (b + 1) * P:2]
                        for kt in range(K_TILES):
                            nc.tensor.matmul(
                                mm_ps[:, eo * MAX_NODES:(eo + 1) * MAX_NODES],
                                lhsT=lhsT_cols[:, kt, :],
                                rhs=zT[:, kt, :],
                                start=(kt == 0),
                                stop=(kt == K_TILES - 1),
                            )
                    nc.scalar.activation(
                        out=o_t[:, b, :],
                        in_=mm_ps,
                        func=mybir.ActivationFunctionType.Sigmoid,
                    )
                nc.sync.dma_start(out=out_r[g], in_=o_t)

    nc.compile()
    return nc


def _get_nc():
    global _NC
    if _NC is None:
        _NC = _build_bass()
    return _NC


def kernel(z, batch, num_graphs, max_nodes):
    global _last_results
    z = np.ascontiguousarray(np.asarray(z), dtype=np.float32)
    batch = np.asarray(batch)
    G = int(num_graphs)
    N = int(max_nodes)
    n_total, d = z.shape
    assert (G, N, d, n_total) == (NUM_GRAPHS, MAX_NODES, LATENT_DIM,
                                  NUM_GRAPHS * MAX_NODES), "hardcoded shapes"

    # Fast path: every graph has exactly max_nodes contiguous nodes.
    expected_batch = (np.arange(n_total) // N).astype(batch.dtype)
    dense = np.array_equal(batch, expected_batch)
    if dense:
        z_full = z
        mask2d = None
    else:
        # General ragged path: scatter into zero-padded [G, N, d] on host,
        # run the same device kernel, then zero out masked positions.
        counts = np.bincount(batch, minlength=G)
        starts = np.concatenate([[0], np.cumsum(counts)[:-1]])
        pos = np.arange(n_total) - starts[batch]
        z_pad = np.zeros((G, N, d), np.float32)
        valid = np.zeros((G, N), bool)
        z_pad[batch, pos] = z
        valid[batch, pos] = True
        z_full = z_pad.reshape(G * N, d)
        mask2d = valid[:, :, None] & valid[:, None, :]

    nc = _get_nc()
    rows = G_PER_CORE * MAX_NODES
    in_maps = [
        {"z": z_full[c * rows:(c + 1) * rows]} for c in range(N_CORES)
    ]
    _last_results = run_bass_kernel_spmd(
        nc, in_maps, core_ids=list(range(N_CORES))
    )
    out = np.concatenate(
        [r["out"] for r in _last_results.results], axis=0
    ).astype(np.float32)

    if mask2d is not None:
        out = np.where(mask2d, out, np.float32(0.0))
    return out
